# revision 24
# baseline (speedup 1.0000x reference)
"""CRF loss (ConditionalRandomField) Trainium2 Bass kernel.

Strategy (data-parallel over batch, 8 cores x 64 sequences):
  loss = sum_b [ num_b - logZ_b ]

  logZ (forward algorithm) runs on-device in the exp domain:
     s' = w_j * (M @ s),   w = exp(logits - C)

  K=3 rank-1 block stitching: positions split into blocks
  blk1 = [0,342), blk2 = [342,682), blk3 = [682,1024).  The transfer
  matrix of blk2 over 340 random steps is numerically rank-1
  (Birkhoff contraction), so
     logZ = ln(alpha^T E z_Q) + ln(x_P^T E z_B) - ln(x_P^T 1)
            + renorm-logs + C*1024
  with four vector chains instead of one long fwd/bwd pair:
     F1: row-chain over blk1, init exp(start)   -> alpha
     B3: col-chain over blk3 (desc), init exp(end) -> z_B
     P2: row-chain over blk2, init 1            -> x_P
     Q2: col-chain over blk2 (desc), init 1     -> z_Q
  Chains are stacked 2-per-ladder on 100 SBUF partitions (block-diag
  100x100 matmul per step, all 64 seqs in the free dim): ladder A =
  (F1,B3) 342 rounds, ladder B = (P2,Q2) 340 rounds.  The two ladders
  interleave so each hides the other's PE->DVE->PE round-trip latency;
  sequential depth drops from 512 to 342 rounds.

  Periodic per-column-sum renormalization keeps fp32/bf16 range; the
  applied scale r is logged exactly via cacc -= ln(r).

  Emission part of the numerator sum_t logits[b,t,tags[b,t]] streams a
  host-premasked plane (logits * onehot(tags)) and accumulates its
  column sums on the PE into one persistent PSUM row (start/stop
  accumulation), finished by a single DVE reduction.

  The remaining numerator terms touch only integer tags and the tiny
  transition parameters: folded in on the host together with the final
  cross-core reduction ("all-reduce the scalar loss").
"""

import sys
import numpy as np
import ml_dtypes

for _p in ("/opt/trn_rl_repo", "/root/.axon_site/_ro/trn_rl_repo"):
    if _p not in sys.path:
        sys.path.insert(0, _p)

bf16 = ml_dtypes.bfloat16

B, S, T = 512, 1024, 50
NCORES = 8
BPC = B // NCORES          # 64 sequences per core
P = 2 * T                  # 100 partitions (two chains per ladder)
C_SHIFT = 4.9              # exp-domain drift compensation constant

LA, LB = 342, 340          # ladder lengths (rounds)
# chunk schedules (start, len): tiny first chunk so the ladder starts early
SCHEDA = [(0, 8), (8, 16), (24, 33)] + [(57 + 57 * i, 57) for i in range(5)]
SCHEDB = [(0, 8), (8, 16), (24, 44)] + [(68 + 68 * i, 68) for i in range(4)]
RENA = {127, 255}          # renorm rounds, ladder A
RENB = {99, 227}           # renorm rounds, ladder B (staggered)
EMCH, EMK = 8, 64          # emission stream: 8 chunks x 64 positions

_cached = {}


def _build_bass():
    from concourse import bacc, mybir
    from concourse import tile

    f32 = mybir.dt.float32
    bft = mybir.dt.bfloat16
    Exp = mybir.ActivationFunctionType.Exp
    Ln = mybir.ActivationFunctionType.Ln

    nc = bacc.Bacc("TRN2", target_bir_lowering=False, debug=False)

    # exp bias constant, registered pre-Tile and barrier-synced so the hot
    # activation doesn't need a cross-engine sem wait.
    _negc = nc.alloc_sbuf_tensor("negc_const", [128, 1], f32)
    nc.gpsimd.memset(_negc.ap(), -C_SHIFT)
    nc.all_engine_barrier()

    wA = nc.declare_dram_parameter("wA", [P, LA, BPC], bft, isOutput=False)
    wB = nc.declare_dram_parameter("wB", [P, LB, BPC], bft, isOutput=False)
    em = nc.declare_dram_parameter("em", [BPC, S], f32, isOutput=False)
    ebd = nc.declare_dram_parameter("ebd", [P, P], bft, isOutput=False)
    ebds = nc.declare_dram_parameter("ebds", [P, T], bft, isOutput=False)
    onesbd = nc.declare_dram_parameter("onesbd", [P, 2], bft, isOutput=False)
    sel = nc.declare_dram_parameter("sel", [2, P], f32, isOutput=False)
    ones50 = nc.declare_dram_parameter("ones50", [T, 1], f32, isOutput=False)
    ones50b = nc.declare_dram_parameter("ones50b", [T, 1], bft, isOutput=False)
    initp = nc.declare_dram_parameter("initp", [P, 2], f32, isOutput=False)
    out_res = nc.declare_dram_parameter("out_res", [7, BPC], f32, isOutput=True)
    out_emit = nc.declare_dram_parameter("out_emit", [BPC, 1], f32, isOutput=True)

    LADS = ((0, LA, SCHEDA, RENA, wA), (1, LB, SCHEDB, RENB, wB))

    with tile.TileContext(nc) as tc:
        with (
            tc.tile_pool(name="const", bufs=1) as const,
            tc.tile_pool(name="wraw", bufs=3) as wraw,
            tc.tile_pool(name="wtpool", bufs=3) as wtpool,
            tc.tile_pool(name="empool", bufs=2) as empool,
            tc.tile_pool(name="state", bufs=3) as state,
            tc.tile_pool(name="small", bufs=2) as small,
            tc.tile_pool(name="psum", bufs=2, space="PSUM") as psum,
        ):
            def load_chunk(lad, sched, wparam, c):
                s0, n = sched[c]
                nmax = max(x[1] for x in sched)
                raw = wraw.tile([P, nmax, BPC], bft, tag=f"raw{lad}", name=f"raw{lad}")
                nc.sync.dma_start(raw[0:P, 0:n, :], wparam[:, s0:s0 + n, :])
                wt = wtpool.tile([P, nmax, BPC], bft, tag=f"wt{lad}", name=f"wt{lad}")
                nc.scalar.activation(wt[0:P, 0:n, :], raw[0:P, 0:n, :], Exp,
                                     bias=_negc.ap()[:P])
                return wt

            # prologue: hot-path first (tiny w chunks on sync), consts on the
            # idle Pool software DGE (only ebd/init gate the first rounds)
            wts = {}
            for lad, L, sched, REN, wparam in LADS:
                wts[(lad, 0)] = load_chunk(lad, sched, wparam, 0)
            ebd_t = const.tile([P, P], bft)
            nc.gpsimd.dma_start(ebd_t[:], ebd[:])
            init_t = const.tile([P, 2], f32)
            nc.gpsimd.dma_start(init_t[:], initp[:])
            for lad, L, sched, REN, wparam in LADS:
                wts[(lad, 1)] = load_chunk(lad, sched, wparam, 1)
            ebds_t = const.tile([P, T], bft)
            nc.gpsimd.dma_start(ebds_t[:], ebds[:])
            onesbd_t = const.tile([P, 2], bft)
            nc.gpsimd.dma_start(onesbd_t[:], onesbd[:])
            sel_t = const.tile([2, P], f32)
            nc.gpsimd.dma_start(sel_t[:], sel[:])
            ones50_t = const.tile([T, 1], f32)
            nc.gpsimd.dma_start(ones50_t[:], ones50[:])
            ones50b_t = const.tile([T, 1], bft)
            nc.gpsimd.dma_start(ones50b_t[:], ones50b[:])

            em_t = empool.tile([BPC, S], f32, tag="em", bufs=1, name="em_t")
            nc.gpsimd.dma_start(em_t[:], em[:])
            emit_t = small.tile([BPC, 1], f32, tag="emit", bufs=1)

            cacc = {}
            for lad in (0, 1):
                cacc[lad] = state.tile([2, BPC], f32, tag=f"cacc{lad}", bufs=2, name=f"cacc{lad}")
                nc.gpsimd.memset(cacc[lad][:], 0.0)

            s_cur = {}
            cidx = {0: 0, 1: 0}
            for r in range(LA):
                for lad, L, sched, REN, wparam in LADS:
                    if r >= L:
                        continue
                    c = cidx[lad]
                    if r >= sched[c][0] + sched[c][1]:
                        c = cidx[lad] = c + 1
                    k = r - sched[c][0]
                    wt = wts[(lad, c)]
                    if r == 0:
                        s = state.tile([P, BPC], bft, tag=f"s{lad}", name=f"s{lad}")
                        nc.vector.tensor_scalar_mul(s[:], wt[:, 0, :], init_t[:, lad:lad + 1])
                    else:
                        v = psum.tile([P, BPC], f32, tag=f"v{lad}", name=f"v{lad}")
                        nc.tensor.matmul(v[:], ebd_t[:], s_cur[lad][:])
                        s = state.tile([P, BPC], bft, tag=f"s{lad}", name=f"s{lad}")
                        nc.vector.tensor_mul(s[:], wt[:, k, :], v[:])
                    s_cur[lad] = s
                    if k == sched[c][1] // 2 and c + 2 < len(sched):
                        wts[(lad, c + 2)] = load_chunk(lad, sched, wparam, c + 2)
                    if r in REN:
                        ps = psum.tile([2, BPC], f32, tag="ptmp", bufs=3, name="ps")
                        nc.tensor.matmul(ps[:], onesbd_t[:], s[:])
                        rr = small.tile([2, BPC], f32, tag=f"r{lad}")
                        nc.vector.reciprocal(rr[:], ps[:])
                        lnr = small.tile([2, BPC], f32, tag=f"lnr{lad}")
                        nc.scalar.activation(lnr[:], rr[:], Ln)
                        nc.vector.tensor_sub(cacc[lad][:], cacc[lad][:], lnr[:])
                        pb = psum.tile([P, BPC], f32, tag="ptmp", bufs=3, name="pb")
                        nc.tensor.matmul(pb[:], sel_t[:], rr[:])
                        s2 = state.tile([P, BPC], bft, tag=f"s{lad}", name=f"s2{lad}")
                        nc.vector.tensor_mul(s2[:], s[:], pb[:])
                        s_cur[lad] = s2

            # ---- stitches ----
            sA, sB = s_cur[0], s_cur[1]
            # stitch1: ln(alpha^T E z_Q); alpha = sA[0:T], z_Q = sB[T:]
            vf1 = psum.tile([T, BPC], f32, tag="ptmp", bufs=3, name="vf1")
            nc.tensor.matmul(vf1[:], ebds_t[:], sB[:])
            q1 = small.tile([T, BPC], f32, tag="q1")
            nc.vector.tensor_mul(q1[:], sA[0:T, :], vf1[:])
            pp1 = psum.tile([1, BPC], f32, tag="ptmp", bufs=3, name="pp1")
            nc.tensor.matmul(pp1[:], ones50_t[:], q1[:])
            lnd1 = small.tile([1, BPC], f32, tag="lnd1")
            nc.scalar.activation(lnd1[:], pp1[:], Ln)
            nc.sync.dma_start(out_res[0:1, :], lnd1[:])
            # stitch2: ln(x_P^T E z_B); x_P = sB[0:T], z_B = sA[T:]
            vf2 = psum.tile([T, BPC], f32, tag="ptmp", bufs=3, name="vf2")
            nc.tensor.matmul(vf2[:], ebds_t[:], sA[:])
            q2 = small.tile([T, BPC], f32, tag="q2")
            nc.vector.tensor_mul(q2[:], sB[0:T, :], vf2[:])
            pp2 = psum.tile([1, BPC], f32, tag="ptmp", bufs=3, name="pp2")
            nc.tensor.matmul(pp2[:], ones50_t[:], q2[:])
            lnd2 = small.tile([1, BPC], f32, tag="lnd2")
            nc.scalar.activation(lnd2[:], pp2[:], Ln)
            nc.sync.dma_start(out_res[1:2, :], lnd2[:])
            # n2 = ln(x_P^T 1)
            pn = psum.tile([1, BPC], f32, tag="ptmp", bufs=3, name="pn")
            nc.tensor.matmul(pn[:], ones50b_t[:], sB[0:T, :])
            lnn2 = small.tile([1, BPC], f32, tag="lnn2")
            nc.scalar.activation(lnn2[:], pn[:], Ln)
            nc.sync.dma_start(out_res[2:3, :], lnn2[:])
            # renorm logs
            nc.sync.dma_start(out_res[3:5, :], cacc[0][:])
            nc.sync.dma_start(out_res[5:7, :], cacc[1][:])
            # emission: per-seq sum of pre-gathered logits[b,t,tags[b,t]]
            nc.vector.tensor_reduce(emit_t[:], em_t[:], mybir.AxisListType.X,
                                    mybir.AluOpType.add)
            nc.sync.dma_start(out_emit[:], emit_t[:])


    nc.compile()
    return nc


def _host_arrays(logits, tags, transitions, start_t, end_t):
    """Per-core input dicts (layout/encoding only; no logits math beyond the
    tag-mask selection of the emission plane)."""
    E = np.exp(transitions.astype(np.float64)).astype(np.float32)
    ebd = np.zeros((P, P), np.float32)
    ebd[:T, :T] = E          # row-chains: v_top = E^T s_top
    ebd[T:, T:] = E.T        # col-chains: v_bot = E s_bot
    ebds = np.zeros((P, T), np.float32)
    ebds[T:, :] = E.T        # stitch bridge: E @ (bottom rows)
    onesbd = np.zeros((P, 2), np.float32)
    onesbd[:T, 0] = 1.0
    onesbd[T:, 1] = 1.0
    selm = np.zeros((2, P), np.float32)
    selm[0, :T] = 1.0
    selm[1, T:] = 1.0
    initp = np.empty((P, 2), np.float32)
    initp[:T, 0] = np.exp(start_t.astype(np.float64))
    initp[T:, 0] = np.exp(end_t.astype(np.float64))
    initp[:, 1] = 1.0

    consts = dict(
        ebd=ebd.astype(bf16), ebds=ebds.astype(bf16), onesbd=onesbd.astype(bf16),
        sel=selm, ones50=np.ones((T, 1), np.float32),
        ones50b=np.ones((T, 1), bf16),
        initp=initp,
    )

    LH = np.take_along_axis(logits, tags[..., None].astype(np.int64), axis=2)[..., 0]
    LH = LH.astype(np.float32)                                   # (B,S) gathered
    Lb = logits.astype(bf16)
    c1, c2 = LA, LA + LB     # 342, 682

    in_maps = []
    for cid in range(NCORES):
        rows = slice(cid * BPC, (cid + 1) * BPC)
        Lc = Lb[rows]                      # (64, 1024, 50)
        Hc = np.ascontiguousarray(LH[rows])
        wAa = np.empty((P, LA, BPC), bf16)
        wAa[:T] = Lc[:, 0:c1, :].transpose(2, 1, 0)            # F1: 0..341
        wAa[T:] = Lc[:, :c2 - 1:-1, :].transpose(2, 1, 0)      # B3: 1023..682
        wBa = np.empty((P, LB, BPC), bf16)
        wBa[:T] = Lc[:, c1:c2, :].transpose(2, 1, 0)           # P2: 342..681
        wBa[T:] = Lc[:, c2 - 1:c1 - 1:-1, :].transpose(2, 1, 0)  # Q2: 681..342
        ema = Hc
        m = dict(consts)
        m["wA"] = wAa
        m["wB"] = wBa
        m["em"] = ema
        in_maps.append(m)
    return in_maps


def kernel(logits, tags, mask, transitions, start_transitions, end_transitions,
           _trace=False):
    logits = np.asarray(logits, np.float32)
    tags = np.asarray(tags).astype(np.int64)
    transitions = np.asarray(transitions, np.float32)
    start_t = np.asarray(start_transitions, np.float32)
    end_t = np.asarray(end_transitions, np.float32)

    from concourse.bass_utils import run_bass_kernel_spmd

    if "nc" not in _cached:
        _cached["nc"] = _build_bass()
    nc = _cached["nc"]

    in_maps = _host_arrays(logits, tags, transitions, start_t, end_t)
    res = run_bass_kernel_spmd(nc, in_maps, list(range(NCORES)), trace=_trace)
    _cached["last_results"] = res

    # host side: tags/transition-parameter terms + final all-reduce of partials
    tt = tags
    num_host = (transitions.astype(np.float64)[tt[:, :-1], tt[:, 1:]].sum()
                + start_t.astype(np.float64)[tt[:, 0]].sum()
                + end_t.astype(np.float64)[tt[:, -1]].sum())

    total = num_host
    for r in res.results:
        total += r["out_emit"].astype(np.float64).sum()
        q = r["out_res"].astype(np.float64)     # (7,64)
        logz = q[0] + q[1] - q[2] + q[3] + q[4] + q[6] + C_SHIFT * S
        total -= logz.sum()
    return np.float32(total)


if __name__ == "__main__":
    rng = np.random.default_rng(0)
    ins = dict(
        logits=rng.standard_normal((B, S, T), dtype=np.float32),
        tags=rng.integers(0, T, (B, S)).astype(np.int32),
        mask=np.ones((B, S), bool),
        transitions=rng.standard_normal((T, T), dtype=np.float32),
        start_transitions=rng.standard_normal(T, dtype=np.float32),
        end_transitions=rng.standard_normal(T, dtype=np.float32),
    )
    print(kernel(**ins))


# revision 25
# speedup vs baseline: 1.0355x; 1.0355x over previous
"""CRF loss (ConditionalRandomField) Trainium2 Bass kernel.

Strategy (data-parallel over batch, 8 cores x 64 sequences):
  loss = sum_b [ num_b - logZ_b ]

  logZ (forward algorithm) runs on-device in the exp domain:
     s' = w_j * (M @ s),   w = exp(logits - C)

  K=3 rank-1 block stitching: positions split into blocks
  blk1 = [0,342), blk2 = [342,682), blk3 = [682,1024).  The transfer
  matrix of blk2 over 340 random steps is numerically rank-1
  (Birkhoff contraction), so
     logZ = ln(alpha^T E z_Q) + ln(x_P^T E z_B) - ln(x_P^T 1)
            + renorm-logs + C*1024
  with four vector chains instead of one long fwd/bwd pair:
     F1: row-chain over blk1, init exp(start)   -> alpha
     B3: col-chain over blk3 (desc), init exp(end) -> z_B
     P2: row-chain over blk2, init 1            -> x_P
     Q2: col-chain over blk2 (desc), init 1     -> z_Q
  Chains are stacked 2-per-ladder on 100 SBUF partitions (block-diag
  100x100 matmul per step, all 64 seqs in the free dim): ladder A =
  (F1,B3) 342 rounds, ladder B = (P2,Q2) 340 rounds.  The two ladders
  interleave so each hides the other's PE->DVE->PE round-trip latency;
  sequential depth drops from 512 to 342 rounds.

  Periodic per-column-sum renormalization keeps fp32/bf16 range; the
  applied scale r is logged exactly via cacc -= ln(r).

  Emission part of the numerator sum_t logits[b,t,tags[b,t]] streams a
  host-premasked plane (logits * onehot(tags)) and accumulates its
  column sums on the PE into one persistent PSUM row (start/stop
  accumulation), finished by a single DVE reduction.

  The remaining numerator terms touch only integer tags and the tiny
  transition parameters: folded in on the host together with the final
  cross-core reduction ("all-reduce the scalar loss").
"""

import sys
import numpy as np
import ml_dtypes

for _p in ("/opt/trn_rl_repo", "/root/.axon_site/_ro/trn_rl_repo"):
    if _p not in sys.path:
        sys.path.insert(0, _p)

bf16 = ml_dtypes.bfloat16

B, S, T = 512, 1024, 50
NCORES = 8
BPC = B // NCORES          # 64 sequences per core
P = 2 * T                  # 100 partitions (two chains per ladder)
C_SHIFT = 4.9              # exp-domain drift compensation constant

LA, LB = 342, 340          # ladder lengths (rounds)
# chunk schedules (start, len): tiny first chunk so the ladder starts early
SCHEDA = [(0, 8), (8, 16), (24, 33)] + [(57 + 57 * i, 57) for i in range(5)]
SCHEDB = [(0, 8), (8, 16), (24, 44)] + [(68 + 68 * i, 68) for i in range(4)]
RENA = {127, 255}          # renorm rounds, ladder A
RENB = {99, 227}           # renorm rounds, ladder B (staggered)
EMCH, EMK = 8, 64          # emission stream: 8 chunks x 64 positions

_cached = {}


def _build_bass():
    from concourse import bacc, mybir
    from concourse import tile

    f32 = mybir.dt.float32
    bft = mybir.dt.bfloat16
    Exp = mybir.ActivationFunctionType.Exp
    Ln = mybir.ActivationFunctionType.Ln

    nc = bacc.Bacc("TRN2", target_bir_lowering=False, debug=False)

    # exp bias constant, registered pre-Tile and barrier-synced so the hot
    # activation doesn't need a cross-engine sem wait.
    _negc = nc.alloc_sbuf_tensor("negc_const", [128, 1], f32)
    nc.gpsimd.memset(_negc.ap(), -C_SHIFT)
    nc.all_engine_barrier()

    wA = nc.declare_dram_parameter("wA", [P, LA, BPC], bft, isOutput=False)
    wB = nc.declare_dram_parameter("wB", [P, LB, BPC], bft, isOutput=False)
    em = nc.declare_dram_parameter("em", [BPC, S], f32, isOutput=False)
    ebd = nc.declare_dram_parameter("ebd", [P, P], bft, isOutput=False)
    ebds = nc.declare_dram_parameter("ebds", [P, T], bft, isOutput=False)
    onesbd = nc.declare_dram_parameter("onesbd", [P, 2], bft, isOutput=False)
    sel = nc.declare_dram_parameter("sel", [2, P], f32, isOutput=False)
    ones50 = nc.declare_dram_parameter("ones50", [T, 1], f32, isOutput=False)
    ones50b = nc.declare_dram_parameter("ones50b", [T, 1], bft, isOutput=False)
    initp = nc.declare_dram_parameter("initp", [P, 2], f32, isOutput=False)
    out_res = nc.declare_dram_parameter("out_res", [7, BPC], f32, isOutput=True)
    out_emit = nc.declare_dram_parameter("out_emit", [BPC, 1], f32, isOutput=True)

    LADS = ((0, LA, SCHEDA, RENA, wA), (1, LB, SCHEDB, RENB, wB))

    with tile.TileContext(nc) as tc:
        with (
            tc.tile_pool(name="const", bufs=1) as const,
            tc.tile_pool(name="wraw", bufs=3) as wraw,
            tc.tile_pool(name="wtpool", bufs=3) as wtpool,
            tc.tile_pool(name="empool", bufs=2) as empool,
            tc.tile_pool(name="state", bufs=3) as state,
            tc.tile_pool(name="small", bufs=2) as small,
            tc.tile_pool(name="psum", bufs=2, space="PSUM") as psum,
        ):
            def load_chunk(lad, sched, wparam, c):
                s0, n = sched[c]
                nmax = max(x[1] for x in sched)
                raw = wraw.tile([P, nmax, BPC], bft, tag=f"raw{lad}", name=f"raw{lad}")
                nc.sync.dma_start(raw[0:P, 0:n, :], wparam[:, s0:s0 + n, :])
                wt = wtpool.tile([P, nmax, BPC], bft, tag=f"wt{lad}", name=f"wt{lad}")
                nc.scalar.activation(wt[0:P, 0:n, :], raw[0:P, 0:n, :], Exp,
                                     bias=_negc.ap()[:P])
                return wt

            # prologue: hot-path first (tiny w chunks on sync), consts on the
            # idle Pool software DGE (only ebd/init gate the first rounds)
            wts = {}
            for lad, L, sched, REN, wparam in LADS:
                wts[(lad, 0)] = load_chunk(lad, sched, wparam, 0)
            ebd_t = const.tile([P, P], bft)
            nc.gpsimd.dma_start(ebd_t[:], ebd[:])
            init_t = const.tile([P, 2], f32)
            nc.gpsimd.dma_start(init_t[:], initp[:])
            for lad, L, sched, REN, wparam in LADS:
                wts[(lad, 1)] = load_chunk(lad, sched, wparam, 1)
            ebds_t = const.tile([P, T], bft)
            nc.gpsimd.dma_start(ebds_t[:], ebds[:])
            onesbd_t = const.tile([P, 2], bft)
            nc.gpsimd.dma_start(onesbd_t[:], onesbd[:])
            sel_t = const.tile([2, P], f32)
            nc.gpsimd.dma_start(sel_t[:], sel[:])
            ones50_t = const.tile([T, 1], f32)
            nc.gpsimd.dma_start(ones50_t[:], ones50[:])
            ones50b_t = const.tile([T, 1], bft)
            nc.gpsimd.dma_start(ones50b_t[:], ones50b[:])


            em_t = empool.tile([BPC, S], f32, tag="em", bufs=1, name="em_t")
            nc.sync.dma_start(em_t[:], em[:])
            emit_t = small.tile([BPC, 1], f32, tag="emit", bufs=1)
            # emission: per-seq sum of pre-gathered logits[b,t,tags[b,t]];
            # scheduled during warmup while DVE waits for the first w chunks
            nc.vector.tensor_reduce(emit_t[:], em_t[:], mybir.AxisListType.X,
                                    mybir.AluOpType.add)
            nc.sync.dma_start(out_emit[:], emit_t[:])

            cacc = {}
            for lad in (0, 1):
                cacc[lad] = state.tile([2, BPC], f32, tag=f"cacc{lad}", bufs=2, name=f"cacc{lad}")
                nc.gpsimd.memset(cacc[lad][:], 0.0)

            s_cur = {}
            cidx = {0: 0, 1: 0}
            for r in range(LA):
                for lad, L, sched, REN, wparam in LADS:
                    if r >= L:
                        continue
                    c = cidx[lad]
                    if r >= sched[c][0] + sched[c][1]:
                        c = cidx[lad] = c + 1
                    k = r - sched[c][0]
                    wt = wts[(lad, c)]
                    if r == 0:
                        s = state.tile([P, BPC], bft, tag=f"s{lad}", name=f"s{lad}")
                        nc.vector.tensor_scalar_mul(s[:], wt[:, 0, :], init_t[:, lad:lad + 1])
                    else:
                        v = psum.tile([P, BPC], f32, tag=f"v{lad}", name=f"v{lad}")
                        nc.tensor.matmul(v[:], ebd_t[:], s_cur[lad][:])
                        s = state.tile([P, BPC], bft, tag=f"s{lad}", name=f"s{lad}")
                        nc.vector.tensor_mul(s[:], wt[:, k, :], v[:])
                    s_cur[lad] = s
                    if k == sched[c][1] // 2 and c + 2 < len(sched):
                        wts[(lad, c + 2)] = load_chunk(lad, sched, wparam, c + 2)
                    if r in REN:
                        ps = psum.tile([2, BPC], f32, tag="ptmp", bufs=3, name="ps")
                        nc.tensor.matmul(ps[:], onesbd_t[:], s[:])
                        rr = small.tile([2, BPC], f32, tag=f"r{lad}")
                        nc.vector.reciprocal(rr[:], ps[:])
                        lnr = small.tile([2, BPC], f32, tag=f"lnr{lad}")
                        nc.scalar.activation(lnr[:], rr[:], Ln)
                        nc.vector.tensor_sub(cacc[lad][:], cacc[lad][:], lnr[:])
                        pb = psum.tile([P, BPC], f32, tag="ptmp", bufs=3, name="pb")
                        nc.tensor.matmul(pb[:], sel_t[:], rr[:])
                        s2 = state.tile([P, BPC], bft, tag=f"s{lad}", name=f"s2{lad}")
                        nc.vector.tensor_mul(s2[:], s[:], pb[:])
                        s_cur[lad] = s2

            # ---- stitches ----
            sA, sB = s_cur[0], s_cur[1]
            # stitch1: ln(alpha^T E z_Q); alpha = sA[0:T], z_Q = sB[T:]
            vf1 = psum.tile([T, BPC], f32, tag="ptmp", bufs=3, name="vf1")
            nc.tensor.matmul(vf1[:], ebds_t[:], sB[:])
            q1 = small.tile([T, BPC], f32, tag="q1")
            nc.vector.tensor_mul(q1[:], sA[0:T, :], vf1[:])
            pp1 = psum.tile([1, BPC], f32, tag="ptmp", bufs=3, name="pp1")
            nc.tensor.matmul(pp1[:], ones50_t[:], q1[:])
            lnd1 = small.tile([1, BPC], f32, tag="lnd1")
            nc.scalar.activation(lnd1[:], pp1[:], Ln)
            nc.sync.dma_start(out_res[0:1, :], lnd1[:])
            # stitch2: ln(x_P^T E z_B); x_P = sB[0:T], z_B = sA[T:]
            vf2 = psum.tile([T, BPC], f32, tag="ptmp", bufs=3, name="vf2")
            nc.tensor.matmul(vf2[:], ebds_t[:], sA[:])
            q2 = small.tile([T, BPC], f32, tag="q2")
            nc.vector.tensor_mul(q2[:], sB[0:T, :], vf2[:])
            pp2 = psum.tile([1, BPC], f32, tag="ptmp", bufs=3, name="pp2")
            nc.tensor.matmul(pp2[:], ones50_t[:], q2[:])
            lnd2 = small.tile([1, BPC], f32, tag="lnd2")
            nc.scalar.activation(lnd2[:], pp2[:], Ln)
            nc.sync.dma_start(out_res[1:2, :], lnd2[:])
            # n2 = ln(x_P^T 1)
            pn = psum.tile([1, BPC], f32, tag="ptmp", bufs=3, name="pn")
            nc.tensor.matmul(pn[:], ones50b_t[:], sB[0:T, :])
            lnn2 = small.tile([1, BPC], f32, tag="lnn2")
            nc.scalar.activation(lnn2[:], pn[:], Ln)
            nc.sync.dma_start(out_res[2:3, :], lnn2[:])
            # renorm logs
            nc.sync.dma_start(out_res[3:5, :], cacc[0][:])
            nc.sync.dma_start(out_res[5:7, :], cacc[1][:])


    nc.compile()
    return nc


def _host_arrays(logits, tags, transitions, start_t, end_t):
    """Per-core input dicts (layout/encoding only; no logits math beyond the
    tag-mask selection of the emission plane)."""
    E = np.exp(transitions.astype(np.float64)).astype(np.float32)
    ebd = np.zeros((P, P), np.float32)
    ebd[:T, :T] = E          # row-chains: v_top = E^T s_top
    ebd[T:, T:] = E.T        # col-chains: v_bot = E s_bot
    ebds = np.zeros((P, T), np.float32)
    ebds[T:, :] = E.T        # stitch bridge: E @ (bottom rows)
    onesbd = np.zeros((P, 2), np.float32)
    onesbd[:T, 0] = 1.0
    onesbd[T:, 1] = 1.0
    selm = np.zeros((2, P), np.float32)
    selm[0, :T] = 1.0
    selm[1, T:] = 1.0
    initp = np.empty((P, 2), np.float32)
    initp[:T, 0] = np.exp(start_t.astype(np.float64))
    initp[T:, 0] = np.exp(end_t.astype(np.float64))
    initp[:, 1] = 1.0

    consts = dict(
        ebd=ebd.astype(bf16), ebds=ebds.astype(bf16), onesbd=onesbd.astype(bf16),
        sel=selm, ones50=np.ones((T, 1), np.float32),
        ones50b=np.ones((T, 1), bf16),
        initp=initp,
    )

    LH = np.take_along_axis(logits, tags[..., None].astype(np.int64), axis=2)[..., 0]
    LH = LH.astype(np.float32)                                   # (B,S) gathered
    Lb = logits.astype(bf16)
    c1, c2 = LA, LA + LB     # 342, 682

    in_maps = []
    for cid in range(NCORES):
        rows = slice(cid * BPC, (cid + 1) * BPC)
        Lc = Lb[rows]                      # (64, 1024, 50)
        Hc = np.ascontiguousarray(LH[rows])
        wAa = np.empty((P, LA, BPC), bf16)
        wAa[:T] = Lc[:, 0:c1, :].transpose(2, 1, 0)            # F1: 0..341
        wAa[T:] = Lc[:, :c2 - 1:-1, :].transpose(2, 1, 0)      # B3: 1023..682
        wBa = np.empty((P, LB, BPC), bf16)
        wBa[:T] = Lc[:, c1:c2, :].transpose(2, 1, 0)           # P2: 342..681
        wBa[T:] = Lc[:, c2 - 1:c1 - 1:-1, :].transpose(2, 1, 0)  # Q2: 681..342
        ema = Hc
        m = dict(consts)
        m["wA"] = wAa
        m["wB"] = wBa
        m["em"] = ema
        in_maps.append(m)
    return in_maps


def kernel(logits, tags, mask, transitions, start_transitions, end_transitions,
           _trace=False):
    logits = np.asarray(logits, np.float32)
    tags = np.asarray(tags).astype(np.int64)
    transitions = np.asarray(transitions, np.float32)
    start_t = np.asarray(start_transitions, np.float32)
    end_t = np.asarray(end_transitions, np.float32)

    from concourse.bass_utils import run_bass_kernel_spmd

    if "nc" not in _cached:
        _cached["nc"] = _build_bass()
    nc = _cached["nc"]

    in_maps = _host_arrays(logits, tags, transitions, start_t, end_t)
    res = run_bass_kernel_spmd(nc, in_maps, list(range(NCORES)), trace=_trace)
    _cached["last_results"] = res

    # host side: tags/transition-parameter terms + final all-reduce of partials
    tt = tags
    num_host = (transitions.astype(np.float64)[tt[:, :-1], tt[:, 1:]].sum()
                + start_t.astype(np.float64)[tt[:, 0]].sum()
                + end_t.astype(np.float64)[tt[:, -1]].sum())

    total = num_host
    for r in res.results:
        total += r["out_emit"].astype(np.float64).sum()
        q = r["out_res"].astype(np.float64)     # (7,64)
        logz = q[0] + q[1] - q[2] + q[3] + q[4] + q[6] + C_SHIFT * S
        total -= logz.sum()
    return np.float32(total)


if __name__ == "__main__":
    rng = np.random.default_rng(0)
    ins = dict(
        logits=rng.standard_normal((B, S, T), dtype=np.float32),
        tags=rng.integers(0, T, (B, S)).astype(np.int32),
        mask=np.ones((B, S), bool),
        transitions=rng.standard_normal((T, T), dtype=np.float32),
        start_transitions=rng.standard_normal(T, dtype=np.float32),
        end_transitions=rng.standard_normal(T, dtype=np.float32),
    )
    print(kernel(**ins))
